# revision 15
# baseline (speedup 1.0000x reference)
"""Banded (sliding-window) attention kernel for Trainium2, 8 NeuronCores.

Problem: x[4,4096,1024] -> qkv = x@W_qkv; S = q k^T / sqrt(1024) with causal
window 100 (row i attends j in [i-99, i]); attn = softmax(S); out =
attn @ v @ W_out + b_out.  Returns (out[4,4096,1024], attn[4,4096,4096]).

Sharding: 8 cores = 4 batches x 2 sequence halves (2048 query rows each).
Attention is local (window 100), so each core only needs a 128-token halo
before its segment -- zero collectives.

Algebraic restructure (exact, just associativity):
  S   = (x Wq)(x Wk)^T / sqrt(d) = x (Wq Wk^T / sqrt(d)) x^T = Y x^T,
        with W' = Wq Wk^T/sqrt(d) precomputed on host, Y = x W'.
  out = P (x Wv) Wout + b = P x (Wv Wout) + b = Z^T Wvo + b,
        with Wvo = Wv Wout precomputed on host (f64) and Z = x^T P^T.
This removes the K and V projections entirely: per-core TensorEngine work
drops from ~660k to ~370k cycles.  All matmuls run in float32r (tf32-like,
~1.3e-4 rel err at K=1024, full rate for moving dim >= 256).

Device layout: host passes x twice per core -- feature-major xT [1024, 2176]
(for Y projection and the S rhs window) and row-major xN [2176, 1024] (the
lhsT for Z).  Two phases over 4 double-blocks of 512 query tokens:
  1a: Y proj (N=512) -> per 128-row tile: S[128,256] = Y-slice @ xT-window
      -> masked softmax -> P (DMA'd out as the attention band) -> P^T via
      PE-transpose -> DRAM scratch.
  1b: Z[d,rows] = xN^T P^T -> outT[dout,rows] = Wvo^T Z + b -> DMA out.
Host scatters the band tiles into the full (mostly zero) attn matrix and
transposes outT back to row-major.
"""
import numpy as np

import concourse.bass as bass
import concourse.mybir as mybir
import concourse.tile as tile
from concourse import bacc
from concourse.bass_utils import run_bass_kernel_spmd
from concourse.masks import make_identity

B, S, DIN, DINNER, DOUT = 4, 4096, 1024, 1024, 1024
NCORES = 8
T = 2048          # query rows per core
HALO = 128        # zero/context pad before each segment
TT = T + HALO     # 2176 padded tokens per core
DB = 512          # double-block of query tokens
NDB = T // DB     # 4
WSPAN = DB + HALO  # 640-wide xT window per double-block
KSPAN = 256       # k span each 128-row tile attends to (band in [29+p, 128+p])
NEG = -1.0e30

f32 = mybir.dt.float32
f32r = mybir.dt.float32r


def _build_nc():
    nc = bacc.Bacc("TRN2", target_bir_lowering=False, debug=False)

    xT = nc.declare_dram_parameter("xT", [DIN, TT], f32r, isOutput=False)
    xN = nc.declare_dram_parameter("xN", [TT, DIN], f32r, isOutput=False)
    wqp = nc.declare_dram_parameter("wqp", [DIN, DIN], f32r, isOutput=False)
    wvo = nc.declare_dram_parameter("wvo", [DIN, DOUT], f32r, isOutput=False)
    bout = nc.declare_dram_parameter("bout", [128, 8], f32, isOutput=False)
    m0f = nc.declare_dram_parameter("mask0f", [128, KSPAN], f32, isOutput=False)
    m0 = nc.declare_dram_parameter("mask0", [128, KSPAN], f32, isOutput=False)
    outT = nc.declare_dram_parameter("outT", [DOUT, T], f32, isOutput=True)
    attnb = nc.declare_dram_parameter(
        "attnb", [T // 128, 128, KSPAN], f32, isOutput=True
    )
    ptd = nc.dram_tensor("pt_scratch", [T // 256, 128, 3, 256], f32r)

    with tile.TileContext(nc) as tc:
        with (
            tc.tile_pool(name="always", bufs=1) as pal,
            tc.tile_pool(name="stat", bufs=8) as pstat,
        ):
            bout_sb = pal.tile([128, 8], f32)
            nc.sync.dma_start(out=bout_sb, in_=bout[:])
            m0f_sb = pal.tile([128, KSPAN], f32)
            nc.sync.dma_start(out=m0f_sb, in_=m0f[:])
            m0_sb = pal.tile([128, KSPAN], f32)
            nc.sync.dma_start(out=m0_sb, in_=m0[:])
            ident = pal.tile([128, 128], f32)
            make_identity(nc, ident)
            zero_sb = pal.tile([128, 128], f32)
            nc.vector.memset(zero_sb, 0.0)
            wvo_sb = pal.tile([128, 8, DOUT], f32r)
            pxn = tc.alloc_tile_pool(name="xn", bufs=6)
            xtiles = {}

            def xn_tile(i):
                if i not in xtiles:
                    t_ = pxn.tile([128, DIN], f32r, tag="xn", name=f"xn{i}")
                    nc.sync.dma_start(out=t_, in_=xN[i * 128 : (i + 1) * 128, :])
                    xtiles[i] = t_
                return xtiles[i]

            # ------- phase 1a: Y proj, scores, softmax, P^T to scratch -----
            with (
                tc.tile_pool(name="w1", bufs=1) as pw1,
                tc.tile_pool(name="xw", bufs=2) as pxw,
                tc.tile_pool(name="yt", bufs=2) as pyt,
                tc.tile_pool(name="pp", bufs=3) as pP,
                tc.tile_pool(name="ptsb", bufs=2) as pPT,
                tc.tile_pool(name="ps_proj", bufs=4, space="PSUM") as pps,
                tc.tile_pool(name="ps_s", bufs=2, space="PSUM") as psS,
                tc.tile_pool(name="ps_t", bufs=2, space="PSUM") as psT,
            ):
                wqp_sb = pw1.tile([128, 8, DIN], f32r)
                prev_w = None
                for bd in range(NDB):
                    # xT window: tokens [DB*bd, DB*bd + 640)
                    xw = pxw.tile([128, 8, WSPAN], f32r, tag="xw")
                    if bd == 0:
                        nc.sync.dma_start(
                            out=xw[:, 0, HALO:WSPAN],
                            in_=xT[0:128, HALO:WSPAN],
                        )
                        nc.sync.dma_start(
                            out=wqp_sb[:, 0, :], in_=wqp[0:128, :]
                        )
                        for ko in range(1, 8):
                            nc.sync.dma_start(
                                out=xw[:, ko, :],
                                in_=xT[ko * 128 : (ko + 1) * 128, 0:WSPAN],
                            )
                            nc.sync.dma_start(
                                out=wqp_sb[:, ko, :],
                                in_=wqp[ko * 128 : (ko + 1) * 128, :],
                            )
                        nc.sync.dma_start(
                            out=xw[:, 0, 0:HALO], in_=xT[0:128, 0:HALO]
                        )
                    else:
                        nc.vector.tensor_copy(xw[:, :, 0:HALO], prev_w[:, :, DB:WSPAN])
                        nc.sync.dma_start(
                            out=xw[:, :, HALO:WSPAN],
                            in_=xT[:, DB * bd + HALO : DB * bd + WSPAN].rearrange(
                                "(ko ki) t -> ki ko t", ki=128
                            ),
                        )
                    # Y projection for the 512 query tokens (window cols 128:640)
                    yt = pyt.tile([128, 8, DB], f32r, tag="yt")
                    for m in range(8):
                        ps = pps.tile([128, DB], f32, tag="ps_proj")
                        for c in range(8):
                            nc.tensor.matmul(
                                ps,
                                wqp_sb[:, c, m * 128 : (m + 1) * 128],
                                xw[:, c, HALO:WSPAN],
                                start=(c == 0),
                                stop=(c == 7),
                            )
                        nc.vector.tensor_copy(yt[:, m, :], ps)
                    for h in range(2):  # 256-row halves of the double-block
                        pt_sb = pPT.tile([128, 3, 256], f32r, tag="ptsb")
                        nc.vector.tensor_copy(pt_sb[:, 2, 0:128], zero_sb)
                        nc.vector.tensor_copy(pt_sb[:, 0, 128:256], zero_sb)
                        sps = []
                        for rr in range(2):
                            r = 2 * h + rr  # 128-row tile within double-block
                            sp = psS.tile([128, KSPAN], f32, tag="ps_s")
                            for c in range(8):
                                nc.tensor.matmul(
                                    sp,
                                    yt[:, c, r * 128 : (r + 1) * 128],
                                    xw[:, c, r * 128 : r * 128 + KSPAN],
                                    start=(c == 0),
                                    stop=(c == 7),
                                )
                            sps.append(sp)
                        for rr in range(2):
                            r = 2 * h + rr
                            sp = sps[rr]
                            mask = m0f_sb if (bd == 0 and r == 0) else m0_sb
                            pp = pP.tile([128, KSPAN], f32, tag="pp")
                            nc.vector.tensor_add(pp, sp, mask)
                            nmx = pstat.tile([128, 1], f32, tag="nmx")
                            nc.vector.tensor_reduce(
                                out=nmx,
                                in_=pp,
                                axis=mybir.AxisListType.X,
                                op=mybir.AluOpType.max,
                                negate=True,
                            )
                            sume = pstat.tile([128, 1], f32, tag="sume")
                            nc.scalar.activation(
                                out=pp,
                                in_=pp,
                                func=mybir.ActivationFunctionType.Exp,
                                bias=nmx,
                                scale=1.0,
                                accum_out=sume,
                            )
                            rin = pstat.tile([128, 1], f32, tag="rin")
                            nc.vector.reciprocal(rin, sume)
                            nc.vector.tensor_scalar_mul(pp, pp, rin)
                            nc.sync.dma_start(out=attnb[4 * bd + r], in_=pp)
                            # live chunks of P are always span-chunks 0 and 1
                            for j in (0, 1):
                                tp = psT.tile([128, 128], f32, tag="ps_t")
                                nc.tensor.transpose(
                                    tp, pp[:, j * 128 : (j + 1) * 128], ident
                                )
                                nc.vector.tensor_copy(
                                    pt_sb[:, rr + j, rr * 128 : (rr + 1) * 128], tp
                                )
                        nc.sync.dma_start(out=ptd[2 * bd + h], in_=pt_sb)
                    if bd == 2:
                        for ko in range(8):
                            nc.sync.dma_start(
                                out=wvo_sb[:, ko, :],
                                in_=wvo[ko * 128 : (ko + 1) * 128, :],
                            )
                        for i in range(5):
                            xn_tile(i)
                    prev_w = xw

            # ------- phase 1b: Z = xN^T P^T, outT = Wvo^T Z + b ------------
            with (
                tc.tile_pool(name="ptl", bufs=3) as pPTl,
                tc.tile_pool(name="zt", bufs=2) as pZ,
                tc.tile_pool(name="outsb", bufs=2) as pOut,
                tc.tile_pool(name="ps_z", bufs=3, space="PSUM") as psZ,
                tc.tile_pool(name="ps_u", bufs=2, space="PSUM") as psU,
            ):
                for bd in range(NDB):
                    zt = pZ.tile([128, 8, DB], f32r, tag="zt")
                    for h in range(2):
                        ptl = pPTl.tile([128, 3, 256], f32r, tag="ptl")
                        nc.sync.dma_start(out=ptl, in_=ptd[2 * bd + h])
                        for m in range(8):
                            zp = psZ.tile([128, 256], f32, tag="ps_z")
                            for s_ in range(3):
                                nc.tensor.matmul(
                                    zp,
                                    xn_tile(4 * bd + 2 * h + s_)[
                                        :, m * 128 : (m + 1) * 128
                                    ],
                                    ptl[:, s_, :],
                                    start=(s_ == 0),
                                    stop=(s_ == 2),
                                )
                            nc.vector.tensor_copy(
                                zt[:, m, h * 256 : (h + 1) * 256], zp
                            )
                    outsb = pOut.tile([128, 8, DB], f32, tag="outsb")
                    for m in range(8):
                        up = psU.tile([128, DB], f32, tag="ps_u")
                        for c in range(8):
                            nc.tensor.matmul(
                                up,
                                wvo_sb[:, c, m * 128 : (m + 1) * 128],
                                zt[:, c, :],
                                start=(c == 0),
                                stop=(c == 7),
                            )
                        nc.scalar.activation(
                            out=outsb[:, m, :],
                            in_=up,
                            func=mybir.ActivationFunctionType.Identity,
                            bias=bout_sb[:, m : m + 1],
                            scale=1.0,
                        )
                        nc.sync.dma_start(
                            out=outT[
                                m * 128 : (m + 1) * 128, bd * DB : (bd + 1) * DB
                            ],
                            in_=outsb[:, m, :],
                        )
            pxn.release()

    nc.compile()
    return nc


def _make_masks():
    p = np.arange(128)[:, None]
    f = np.arange(KSPAN)[None, :]
    live = (f >= 29 + p) & (f <= 128 + p)
    m0 = np.where(live, 0.0, NEG).astype(np.float32)
    m0f = np.where(live & (f >= 128), 0.0, NEG).astype(np.float32)
    return m0, m0f


_NC_CACHE = {}


def run_cores(x, W_qkv, W_out, b_out, trace=False):
    x = np.ascontiguousarray(np.asarray(x, dtype=np.float32))
    W_qkv = np.asarray(W_qkv, dtype=np.float32)
    W_out = np.asarray(W_out, dtype=np.float32)
    b_out = np.asarray(b_out, dtype=np.float32)

    scale = DINNER ** -0.5
    Wq = W_qkv[:, :DINNER].astype(np.float64)
    Wk = W_qkv[:, DINNER : 2 * DINNER].astype(np.float64)
    Wv = W_qkv[:, 2 * DINNER :].astype(np.float64)
    wqp = np.ascontiguousarray((Wq @ Wk.T * scale).astype(np.float32))
    wvo = np.ascontiguousarray((Wv @ W_out.astype(np.float64)).astype(np.float32))
    boutt = np.ascontiguousarray(b_out.reshape(8, 128).T)
    m0, m0f = _make_masks()

    if "nc" not in _NC_CACHE:
        _NC_CACHE["nc"] = _build_nc()
    nc = _NC_CACHE["nc"]

    in_maps = []
    for ci in range(NCORES):
        batch, half = divmod(ci, 2)
        seg = x[batch, half * T : half * T + T]
        xNc = np.empty((TT, DIN), np.float32)
        if half == 0:
            xNc[:HALO] = 0.0
        else:
            xNc[:HALO] = x[batch, T - HALO : T]
        xNc[HALO:] = seg
        xTc = np.ascontiguousarray(xNc.T)
        in_maps.append(
            {
                "xT": xTc,
                "xN": xNc,
                "wqp": wqp,
                "wvo": wvo,
                "bout": boutt,
                "mask0f": m0f if half == 0 else m0,
                "mask0": m0,
            }
        )

    res = run_bass_kernel_spmd(nc, in_maps, core_ids=list(range(NCORES)), trace=trace)

    out = np.empty((B, S, DOUT), np.float32)
    attn = np.zeros((B, S, S), np.float32)
    for ci in range(NCORES):
        batch, half = divmod(ci, 2)
        r = res.results[ci]
        out[batch, half * T : (half + 1) * T] = r["outT"].T
        ab = r["attnb"]
        for k in range(T // 128):
            row0 = half * T + 128 * k
            js = half * T + 128 * (k - 1)
            if js < 0:
                attn[batch, row0 : row0 + 128, 0 : js + KSPAN] = ab[k][:, -js:]
            else:
                attn[batch, row0 : row0 + 128, js : js + KSPAN] = ab[k]
    return out, attn, res


def kernel(x, W_qkv, W_out, b_out):
    out, attn, _ = run_cores(x, W_qkv, W_out, b_out, trace=False)
    return out, attn


# revision 16
# speedup vs baseline: 1.0579x; 1.0579x over previous
"""Banded (sliding-window) attention kernel for Trainium2, 8 NeuronCores.

Problem: x[4,4096,1024] -> qkv = x@W_qkv; S = q k^T / sqrt(1024) with causal
window 100 (row i attends j in [i-99, i]); attn = softmax(S); out =
attn @ v @ W_out + b_out.  Returns (out[4,4096,1024], attn[4,4096,4096]).

Sharding: 8 cores = 4 batches x 2 sequence halves (2048 query rows each).
Attention is local (window 100), so each core only needs a 128-token halo
before its segment -- zero collectives.

Algebraic restructure (exact, just associativity):
  S   = (x Wq)(x Wk)^T / sqrt(d) = x (Wq Wk^T / sqrt(d)) x^T = Y x^T,
        with W' = Wq Wk^T/sqrt(d) precomputed on host, Y = x W'.
  out = P (x Wv) Wout + b = P x (Wv Wout) + b = Z^T Wvo + b,
        with Wvo = Wv Wout precomputed on host (f64) and Z = x^T P^T.
This removes the K and V projections entirely: per-core TensorEngine work
drops from ~660k to ~370k cycles.  All matmuls run in float32r (tf32-like,
~1.3e-4 rel err at K=1024, full rate for moving dim >= 256).

Device layout: host passes x twice per core -- feature-major xT [1024, 2176]
(for Y projection and the S rhs window) and row-major xN [2176, 1024] (the
lhsT for Z).  Two phases over 4 double-blocks of 512 query tokens:
  1a: Y proj (N=512) -> per 128-row tile: S[128,256] = Y-slice @ xT-window
      -> masked softmax -> P (DMA'd out as the attention band) -> P^T via
      PE-transpose -> DRAM scratch.
  1b: Z[d,rows] = xN^T P^T -> outT[dout,rows] = Wvo^T Z + b -> DMA out.
Host scatters the band tiles into the full (mostly zero) attn matrix and
transposes outT back to row-major.
"""
import numpy as np

import concourse.bass as bass
import concourse.mybir as mybir
import concourse.tile as tile
from concourse import bacc
from concourse.bass_utils import run_bass_kernel_spmd
from concourse.masks import make_identity

B, S, DIN, DINNER, DOUT = 4, 4096, 1024, 1024, 1024
NCORES = 8
T = 2048          # query rows per core
HALO = 128        # zero/context pad before each segment
TT = T + HALO     # 2176 padded tokens per core
DB = 512          # double-block of query tokens
NDB = T // DB     # 4
WSPAN = DB + HALO  # 640-wide xT window per double-block
KSPAN = 256       # k span each 128-row tile attends to (band in [29+p, 128+p])
NEG = -1.0e30

f32 = mybir.dt.float32
f32r = mybir.dt.float32r


def _build_nc():
    nc = bacc.Bacc("TRN2", target_bir_lowering=False, debug=False)

    xT = nc.declare_dram_parameter("xT", [DIN, TT], f32r, isOutput=False)
    xN = nc.declare_dram_parameter("xN", [TT, DIN], f32r, isOutput=False)
    wqp = nc.declare_dram_parameter("wqp", [DIN, DIN], f32r, isOutput=False)
    wvo = nc.declare_dram_parameter("wvo", [DIN, DOUT], f32r, isOutput=False)
    bout = nc.declare_dram_parameter("bout", [128, 8], f32, isOutput=False)
    m0f = nc.declare_dram_parameter("mask0f", [128, KSPAN], f32, isOutput=False)
    m0 = nc.declare_dram_parameter("mask0", [128, KSPAN], f32, isOutput=False)
    outT = nc.declare_dram_parameter("outT", [DOUT, T], f32, isOutput=True)
    attnb = nc.declare_dram_parameter(
        "attnb", [T // 128, 128, KSPAN], f32, isOutput=True
    )
    ptd = nc.dram_tensor("pt_scratch", [T // 256, 128, 3, 256], f32r)

    with tile.TileContext(nc) as tc:
        with (
            tc.tile_pool(name="always", bufs=1) as pal,
            tc.tile_pool(name="stat", bufs=8) as pstat,
        ):
            bout_sb = pal.tile([128, 8], f32)
            nc.sync.dma_start(out=bout_sb, in_=bout[:])
            m0f_sb = pal.tile([128, KSPAN], f32)
            nc.sync.dma_start(out=m0f_sb, in_=m0f[:])
            m0_sb = pal.tile([128, KSPAN], f32)
            nc.sync.dma_start(out=m0_sb, in_=m0[:])
            ident = pal.tile([128, 128], f32)
            make_identity(nc, ident)
            zero_sb = pal.tile([128, 128], f32)
            nc.vector.memset(zero_sb, 0.0)
            wvo_sb = pal.tile([128, 8, DOUT], f32r)
            pxn = tc.alloc_tile_pool(name="xn", bufs=6)
            xtiles = {}

            def xn_tile(i):
                if i not in xtiles:
                    t_ = pxn.tile([128, DIN], f32r, tag="xn", name=f"xn{i}")
                    nc.sync.dma_start(out=t_, in_=xN[i * 128 : (i + 1) * 128, :])
                    xtiles[i] = t_
                return xtiles[i]

            # ------- phase 1a: Y proj, scores, softmax, P^T to scratch -----
            with (
                tc.tile_pool(name="w1", bufs=1) as pw1,
                tc.tile_pool(name="xw", bufs=2) as pxw,
                tc.tile_pool(name="yt", bufs=2) as pyt,
                tc.tile_pool(name="pp", bufs=5) as pP,
                tc.tile_pool(name="ptsb", bufs=2) as pPT,
                tc.tile_pool(name="ps_proj", bufs=4, space="PSUM") as pps,
                tc.tile_pool(name="ps_s", bufs=2, space="PSUM") as psS,
                tc.tile_pool(name="ps_t", bufs=2, space="PSUM") as psT,
            ):
                wqp_sb = pw1.tile([128, 8, DIN], f32r)
                prev_w = None
                for bd in range(NDB):
                    # xT window: tokens [DB*bd, DB*bd + 640)
                    xw = pxw.tile([128, 8, WSPAN], f32r, tag="xw")
                    if bd == 0:
                        nc.sync.dma_start(
                            out=xw[:, 0, HALO:WSPAN],
                            in_=xT[0:128, HALO:WSPAN],
                        )
                        nc.sync.dma_start(
                            out=wqp_sb[:, 0, :], in_=wqp[0:128, :]
                        )
                        for ko in range(1, 8):
                            nc.sync.dma_start(
                                out=xw[:, ko, :],
                                in_=xT[ko * 128 : (ko + 1) * 128, 0:WSPAN],
                            )
                            nc.sync.dma_start(
                                out=wqp_sb[:, ko, :],
                                in_=wqp[ko * 128 : (ko + 1) * 128, :],
                            )
                        nc.sync.dma_start(
                            out=xw[:, 0, 0:HALO], in_=xT[0:128, 0:HALO]
                        )
                    else:
                        nc.vector.tensor_copy(xw[:, :, 0:HALO], prev_w[:, :, DB:WSPAN])
                        nc.sync.dma_start(
                            out=xw[:, :, HALO:WSPAN],
                            in_=xT[:, DB * bd + HALO : DB * bd + WSPAN].rearrange(
                                "(ko ki) t -> ki ko t", ki=128
                            ),
                        )
                    # Y projection for the 512 query tokens (window cols 128:640)
                    yt = pyt.tile([128, 8, DB], f32r, tag="yt")
                    for m in range(8):
                        ps = pps.tile([128, DB], f32, tag="ps_proj")
                        for c in range(8):
                            nc.tensor.matmul(
                                ps,
                                wqp_sb[:, c, m * 128 : (m + 1) * 128],
                                xw[:, c, HALO:WSPAN],
                                start=(c == 0),
                                stop=(c == 7),
                            )
                        nc.vector.tensor_copy(yt[:, m, :], ps)
                    pts = []
                    for h in range(2):
                        pt_sb = pPT.tile([128, 3, 256], f32r, tag="ptsb", name=f"pt{bd}_{h}")
                        nc.vector.tensor_copy(pt_sb[:, 2, 0:128], zero_sb)
                        nc.vector.tensor_copy(pt_sb[:, 0, 128:256], zero_sb)
                        pts.append(pt_sb)
                    pps_l = []
                    for r in range(4):  # all four S groups first: PE never
                        sp = psS.tile([128, KSPAN], f32, tag="ps_s")
                        for c in range(8):
                            nc.tensor.matmul(
                                sp,
                                yt[:, c, r * 128 : (r + 1) * 128],
                                xw[:, c, r * 128 : r * 128 + KSPAN],
                                start=(c == 0),
                                stop=(c == 7),
                            )
                        mask = m0f_sb if (bd == 0 and r == 0) else m0_sb
                        pp = pP.tile([128, KSPAN], f32, tag="pp", name=f"pp{bd}_{r}")
                        nc.vector.tensor_add(pp, sp, mask)
                        nmx = pstat.tile([128, 1], f32, tag="nmx")
                        nc.vector.tensor_reduce(
                            out=nmx,
                            in_=pp,
                            axis=mybir.AxisListType.X,
                            op=mybir.AluOpType.max,
                            negate=True,
                        )
                        sume = pstat.tile([128, 1], f32, tag="sume")
                        nc.scalar.activation(
                            out=pp,
                            in_=pp,
                            func=mybir.ActivationFunctionType.Exp,
                            bias=nmx,
                            scale=1.0,
                            accum_out=sume,
                        )
                        rin = pstat.tile([128, 1], f32, tag="rin")
                        nc.vector.reciprocal(rin, sume)
                        nc.vector.tensor_scalar_mul(pp, pp, rin)
                        nc.sync.dma_start(out=attnb[4 * bd + r], in_=pp)
                        pps_l.append(pp)
                    for r in range(4):  # transposes after: softmax latency hidden
                        rr = r % 2
                        for j in (0, 1):
                            tp = psT.tile([128, 128], f32, tag="ps_t")
                            nc.tensor.transpose(
                                tp, pps_l[r][:, j * 128 : (j + 1) * 128], ident
                            )
                            nc.vector.tensor_copy(
                                pts[r // 2][:, rr + j, rr * 128 : (rr + 1) * 128], tp
                            )
                    for h in range(2):
                        nc.sync.dma_start(out=ptd[2 * bd + h], in_=pts[h])
                    if bd == 2:
                        for ko in range(8):
                            nc.sync.dma_start(
                                out=wvo_sb[:, ko, :],
                                in_=wvo[ko * 128 : (ko + 1) * 128, :],
                            )
                        for i in range(5):
                            xn_tile(i)
                    prev_w = xw

            # ------- phase 1b: Z = xN^T P^T, outT = Wvo^T Z + b ------------
            with (
                tc.tile_pool(name="ptl", bufs=3) as pPTl,
                tc.tile_pool(name="zt", bufs=2) as pZ,
                tc.tile_pool(name="outsb", bufs=2) as pOut,
                tc.tile_pool(name="ps_z", bufs=3, space="PSUM") as psZ,
                tc.tile_pool(name="ps_u", bufs=2, space="PSUM") as psU,
            ):
                for bd in range(NDB):
                    zt = pZ.tile([128, 8, DB], f32r, tag="zt")
                    for h in range(2):
                        ptl = pPTl.tile([128, 3, 256], f32r, tag="ptl")
                        nc.sync.dma_start(out=ptl, in_=ptd[2 * bd + h])
                        for m in range(8):
                            zp = psZ.tile([128, 256], f32, tag="ps_z")
                            for s_ in range(3):
                                nc.tensor.matmul(
                                    zp,
                                    xn_tile(4 * bd + 2 * h + s_)[
                                        :, m * 128 : (m + 1) * 128
                                    ],
                                    ptl[:, s_, :],
                                    start=(s_ == 0),
                                    stop=(s_ == 2),
                                )
                            nc.vector.tensor_copy(
                                zt[:, m, h * 256 : (h + 1) * 256], zp
                            )
                    outsb = pOut.tile([128, 8, DB], f32, tag="outsb")
                    for m in range(8):
                        up = psU.tile([128, DB], f32, tag="ps_u")
                        for c in range(8):
                            nc.tensor.matmul(
                                up,
                                wvo_sb[:, c, m * 128 : (m + 1) * 128],
                                zt[:, c, :],
                                start=(c == 0),
                                stop=(c == 7),
                            )
                        nc.scalar.activation(
                            out=outsb[:, m, :],
                            in_=up,
                            func=mybir.ActivationFunctionType.Identity,
                            bias=bout_sb[:, m : m + 1],
                            scale=1.0,
                        )
                        nc.sync.dma_start(
                            out=outT[
                                m * 128 : (m + 1) * 128, bd * DB : (bd + 1) * DB
                            ],
                            in_=outsb[:, m, :],
                        )
            pxn.release()

    nc.compile()
    return nc


def _make_masks():
    p = np.arange(128)[:, None]
    f = np.arange(KSPAN)[None, :]
    live = (f >= 29 + p) & (f <= 128 + p)
    m0 = np.where(live, 0.0, NEG).astype(np.float32)
    m0f = np.where(live & (f >= 128), 0.0, NEG).astype(np.float32)
    return m0, m0f


_NC_CACHE = {}


def run_cores(x, W_qkv, W_out, b_out, trace=False):
    x = np.ascontiguousarray(np.asarray(x, dtype=np.float32))
    W_qkv = np.asarray(W_qkv, dtype=np.float32)
    W_out = np.asarray(W_out, dtype=np.float32)
    b_out = np.asarray(b_out, dtype=np.float32)

    scale = DINNER ** -0.5
    Wq = W_qkv[:, :DINNER].astype(np.float64)
    Wk = W_qkv[:, DINNER : 2 * DINNER].astype(np.float64)
    Wv = W_qkv[:, 2 * DINNER :].astype(np.float64)
    wqp = np.ascontiguousarray((Wq @ Wk.T * scale).astype(np.float32))
    wvo = np.ascontiguousarray((Wv @ W_out.astype(np.float64)).astype(np.float32))
    boutt = np.ascontiguousarray(b_out.reshape(8, 128).T)
    m0, m0f = _make_masks()

    if "nc" not in _NC_CACHE:
        _NC_CACHE["nc"] = _build_nc()
    nc = _NC_CACHE["nc"]

    in_maps = []
    for ci in range(NCORES):
        batch, half = divmod(ci, 2)
        seg = x[batch, half * T : half * T + T]
        xNc = np.empty((TT, DIN), np.float32)
        if half == 0:
            xNc[:HALO] = 0.0
        else:
            xNc[:HALO] = x[batch, T - HALO : T]
        xNc[HALO:] = seg
        xTc = np.ascontiguousarray(xNc.T)
        in_maps.append(
            {
                "xT": xTc,
                "xN": xNc,
                "wqp": wqp,
                "wvo": wvo,
                "bout": boutt,
                "mask0f": m0f if half == 0 else m0,
                "mask0": m0,
            }
        )

    res = run_bass_kernel_spmd(nc, in_maps, core_ids=list(range(NCORES)), trace=trace)

    out = np.empty((B, S, DOUT), np.float32)
    attn = np.zeros((B, S, S), np.float32)
    for ci in range(NCORES):
        batch, half = divmod(ci, 2)
        r = res.results[ci]
        out[batch, half * T : (half + 1) * T] = r["outT"].T
        ab = r["attnb"]
        for k in range(T // 128):
            row0 = half * T + 128 * k
            js = half * T + 128 * (k - 1)
            if js < 0:
                attn[batch, row0 : row0 + 128, 0 : js + KSPAN] = ab[k][:, -js:]
            else:
                attn[batch, row0 : row0 + 128, js : js + KSPAN] = ab[k]
    return out, attn, res


def kernel(x, W_qkv, W_out, b_out):
    out, attn, _ = run_cores(x, W_qkv, W_out, b_out, trace=False)
    return out, attn
